# revision 7
# baseline (speedup 1.0000x reference)
"""Trainium2 Bass kernel for a 4-layer "max-tempered" MLP.

Math per layer (BETA=0.2):
    h_next[b,o] = (1-BETA) * sum_i(W[o,i]*h[b,i]) + BETA * max_i(W[o,i]*h[b,i]) + bias[o]
Dims: 1024x256 -> 512 -> 512 -> 512 -> 1, output squeezed to (1024,).

Sharding: data-parallel over batch, 128 rows per core on 8 cores; weights
replicated. The linear term runs on the PE (matmul); the tropical max term
runs on the DVE via tensor_tensor_reduce with the activation row broadcast
across partitions by GpSimd. Layouts keep h^T (feature-on-partition) for
matmul/combine and h natural (batch-on-partition) for the broadcast source.
"""

import os
import sys

import numpy as np

sys.path.insert(0, "/opt/trn_rl_repo")

B, IN, HID = 1024, 256, 512
NCORES, BC = 8, 128
BETA = 0.2
NEG_INIT = -1e30

# set by kernel() when BASS_KERNEL_TRACE=1: (exec_time_ns, results_obj)
LAST_EXEC_NS = None
LAST_RESULTS = None


def build_program(b4_val: float):
    from contextlib import ExitStack

    from concourse import bacc, bass, mybir, tile
    from concourse.masks import make_identity

    f32 = mybir.dt.float32
    A = mybir.AluOpType
    AX = mybir.AxisListType.X

    nc = bacc.Bacc("TRN2", target_bir_lowering=False, debug=False,
                   num_devices=NCORES)

    d_xT = nc.declare_dram_parameter("xT", [IN, BC], f32, isOutput=False)
    d_xn = nc.declare_dram_parameter("xn", [BC, IN], f32, isOutput=False)
    d_w1t = nc.declare_dram_parameter("w1t", [IN, HID], f32, isOutput=False)
    d_w1n = nc.declare_dram_parameter("w1n", [HID, IN], f32, isOutput=False)
    d_w2t = nc.declare_dram_parameter("w2t", [HID, HID], f32, isOutput=False)
    d_w2n = nc.declare_dram_parameter("w2n", [HID, HID], f32, isOutput=False)
    d_w3t = nc.declare_dram_parameter("w3t", [HID, HID], f32, isOutput=False)
    d_w3n = nc.declare_dram_parameter("w3n", [HID, HID], f32, isOutput=False)
    d_b1 = nc.declare_dram_parameter("b1c", [BC, 4], f32, isOutput=False)
    d_b2 = nc.declare_dram_parameter("b2c", [BC, 4], f32, isOutput=False)
    d_b3 = nc.declare_dram_parameter("b3c", [BC, 4], f32, isOutput=False)
    d_w4b = nc.declare_dram_parameter("w4b", [BC, HID], f32, isOutput=False)
    d_w4t = nc.declare_dram_parameter("w4t", [HID, 1], f32, isOutput=False)
    d_out = nc.declare_dram_parameter("out", [BC, 1], f32, isOutput=True)

    with tile.TileContext(nc) as tc, ExitStack() as ctx:
        wts = ctx.enter_context(tc.tile_pool(name="wts", bufs=1))
        work = ctx.enter_context(tc.tile_pool(name="work", bufs=1))
        psum = ctx.enter_context(
            tc.tile_pool(name="psum", bufs=1, space=bass.MemorySpace.PSUM))

        ident = wts.tile([BC, BC], f32, tag="ident", name="ident")
        make_identity(nc, ident[:])

        w1t = [wts.tile([BC, HID], f32, tag=f"w1t{i}", name=f"w1t{i}")
               for i in range(2)]
        w1n = [wts.tile([BC, IN], f32, tag=f"w1n{o}", name=f"w1n{o}")
               for o in range(4)]
        w2t = [wts.tile([BC, HID], f32, tag=f"w2t{i}", name=f"w2t{i}")
               for i in range(4)]
        w2n = [wts.tile([BC, HID], f32, tag=f"w2n{o}", name=f"w2n{o}")
               for o in range(4)]
        w3t = [wts.tile([BC, HID], f32, tag=f"w3t{i}", name=f"w3t{i}")
               for i in range(4)]
        w3n = [wts.tile([BC, HID], f32, tag=f"w3n{o}", name=f"w3n{o}")
               for o in range(4)]
        b1c = wts.tile([BC, 4], f32, tag="b1c", name="b1c")
        b2c = wts.tile([BC, 4], f32, tag="b2c", name="b2c")
        b3c = wts.tile([BC, 4], f32, tag="b3c", name="b3c")
        w4b = wts.tile([BC, HID], f32, tag="w4b", name="w4b")
        w4t = [wts.tile([BC, 1], f32, tag=f"w4t{i}", name=f"w4t{i}")
               for i in range(4)]
        xT = [wts.tile([BC, BC], f32, tag=f"xT{i}", name=f"xT{i}")
              for i in range(2)]
        xn = wts.tile([BC, IN], f32, tag="xn", name="xn")

        for i in range(2):
            nc.sync.dma_start(w1t[i][:], d_w1t[i * BC:(i + 1) * BC, :])
            nc.sync.dma_start(xT[i][:], d_xT[i * BC:(i + 1) * BC, :])
        for o in range(4):
            nc.sync.dma_start(w1n[o][:], d_w1n[o * BC:(o + 1) * BC, :])
            nc.sync.dma_start(w2t[o][:], d_w2t[o * BC:(o + 1) * BC, :])
            nc.sync.dma_start(w2n[o][:], d_w2n[o * BC:(o + 1) * BC, :])
            nc.sync.dma_start(w3t[o][:], d_w3t[o * BC:(o + 1) * BC, :])
            nc.sync.dma_start(w3n[o][:], d_w3n[o * BC:(o + 1) * BC, :])
            nc.sync.dma_start(w4t[o][:], d_w4t[o * BC:(o + 1) * BC, :])
        nc.sync.dma_start(xn[:], d_xn[:])
        nc.sync.dma_start(b1c[:], d_b1[:])
        nc.sync.dma_start(b2c[:], d_b2[:])
        nc.sync.dma_start(b3c[:], d_b3[:])
        nc.sync.dma_start(w4b[:], d_w4b[:])

        def mt_layer(lidx, hT_tiles, h_nat, wt, wn, bias, i_sz):
            """One max-tempered layer; returns the 4 hT tiles of the output."""
            n_it = i_sz // BC
            lins = []
            for o in range(4):
                lin = psum.tile([BC, BC], f32, tag=f"lin{o}",
                                name=f"lin{lidx}_{o}")
                for it in range(n_it):
                    nc.tensor.matmul(
                        lin[:],
                        wt[it][:, o * BC:(o + 1) * BC],
                        hT_tiles[it][:],
                        start=(it == 0),
                        stop=(it == n_it - 1),
                    )
                lins.append(lin)

            MT = [work.tile([BC, BC], f32, tag=f"mt{o}", name=f"mt{lidx}_{o}")
                  for o in range(4)]
            scr = work.tile([BC, i_sz], f32, tag="scr", name=f"scr{lidx}")
            for b in range(BC):
                # broadcast h row b across partitions: one-hot PE matmul
                # out[p,i] = sum_k ident[k,b] * h_nat[k,i] = h_nat[b,i]
                hb = psum.tile([BC, i_sz], f32, tag="hb", bufs=2,
                               name=f"hb{lidx}_{b}")
                sel = ident[:, b:b + 1].broadcast_to((BC, BC))
                nc.tensor.matmul(hb[:], sel, h_nat[:, 0:i_sz],
                                 start=True, stop=True)
                for o in range(4):
                    nc.vector.scalar_tensor_tensor(
                        out=scr[:], in0=wn[o][:], scalar=BETA, in1=hb[:],
                        op0=A.mult, op1=A.mult)
                    nc.vector.tensor_reduce(
                        out=MT[o][:, b:b + 1], in_=scr[:],
                        axis=AX, op=A.max)

            new_hT = []
            for o in range(4):
                hT = work.tile([BC, BC], f32, tag=f"hT{o}", bufs=2,
                               name=f"hT{lidx}_{o}")
                nc.vector.scalar_tensor_tensor(
                    out=hT[:], in0=lins[o][:], scalar=1.0 - BETA,
                    in1=MT[o][:], op0=A.mult, op1=A.add)
                nc.vector.tensor_scalar(
                    out=hT[:], in0=hT[:], scalar1=bias[:, o:o + 1],
                    scalar2=None, op0=A.add)
                new_hT.append(hT)
            return new_hT

        def to_nat(lidx, hT_tiles):
            """Transpose 4 hT tiles [o x b] into h natural [b x 512]."""
            h_nat = work.tile([BC, HID], f32, tag="hnat", bufs=2,
                              name=f"hnat{lidx}")
            for o in range(4):
                tp = psum.tile([BC, BC], f32, tag="tp", bufs=1,
                               name=f"tp{lidx}_{o}")
                nc.tensor.transpose(tp[:], hT_tiles[o][:], ident[:])
                nc.scalar.copy(h_nat[:, o * BC:(o + 1) * BC], tp[:])
            return h_nat

        h1T = mt_layer(1, xT, xn, w1t, w1n, b1c, IN)
        h1n = to_nat(1, h1T)
        h2T = mt_layer(2, h1T, h1n, w2t, w2n, b2c, HID)
        h2n = to_nat(2, h2T)
        h3T = mt_layer(3, h2T, h2n, w3t, w3n, b3c, HID)
        h3n = to_nat(3, h3T)

        # Layer 4 (single output): max term via one TTR with W4 row
        # replicated across partitions (host-precomputed w4b).
        scr4 = work.tile([BC, HID], f32, tag="scr", name="scr4")
        m4 = work.tile([BC, 1], f32, tag="m4", name="m4")
        nc.vector.scalar_tensor_tensor(
            out=scr4[:], in0=h3n[:], scalar=BETA, in1=w4b[:],
            op0=A.mult, op1=A.mult)
        nc.vector.tensor_reduce(out=m4[:], in_=scr4[:], axis=AX, op=A.max)

        lin4 = psum.tile([BC, 1], f32, tag="lin4", name="lin4")
        for it in range(4):
            nc.tensor.matmul(lin4[:], h3T[it][:], w4t[it][:],
                             start=(it == 0), stop=(it == 3))

        oc = work.tile([BC, 1], f32, tag="oc", name="oc")
        nc.vector.scalar_tensor_tensor(
            out=oc[:], in0=lin4[:], scalar=1.0 - BETA, in1=m4[:],
            op0=A.mult, op1=A.add)
        nc.vector.tensor_scalar(
            out=oc[:], in0=oc[:], scalar1=b4_val, scalar2=None, op0=A.add)
        nc.sync.dma_start(d_out[:], oc[:])

    nc.compile()
    return nc


def make_in_maps(x, W1, b1, W2, b2, W3, b3, W4, b4):
    f = np.float32
    c = np.ascontiguousarray
    base = {
        "w1t": c(W1.T, dtype=f), "w1n": c(W1, dtype=f),
        "w2t": c(W2.T, dtype=f), "w2n": c(W2, dtype=f),
        "w3t": c(W3.T, dtype=f), "w3n": c(W3, dtype=f),
        "b1c": c(b1.reshape(4, BC).T, dtype=f),
        "b2c": c(b2.reshape(4, BC).T, dtype=f),
        "b3c": c(b3.reshape(4, BC).T, dtype=f),
        "w4b": c(np.broadcast_to(W4.reshape(1, HID), (BC, HID)), dtype=f),
        "w4t": c(W4.reshape(HID, 1), dtype=f),
    }
    in_maps = []
    for cidx in range(NCORES):
        xs = np.asarray(x[cidx * BC:(cidx + 1) * BC], dtype=f)
        m = dict(base)
        m["xn"] = c(xs)
        m["xT"] = c(xs.T)
        in_maps.append(m)
    return in_maps


def kernel(x, W1, b1, W2, b2, W3, b3, W4, b4):
    global LAST_EXEC_NS, LAST_RESULTS
    from concourse.bass_utils import run_bass_kernel_spmd

    nc = build_program(float(np.asarray(b4).reshape(-1)[0]))
    in_maps = make_in_maps(x, W1, b1, W2, b2, W3, b3, W4, b4)
    trace = os.environ.get("BASS_KERNEL_TRACE", "0") == "1"
    try:
        res = run_bass_kernel_spmd(nc, in_maps, list(range(NCORES)),
                                   trace=trace)
    except ModuleNotFoundError:
        res = run_bass_kernel_spmd(nc, in_maps, list(range(NCORES)),
                                   trace=False)
    LAST_EXEC_NS = res.exec_time_ns
    LAST_RESULTS = res
    out = np.concatenate(
        [np.asarray(res.results[cidx]["out"]).reshape(BC)
         for cidx in range(NCORES)])
    return out.astype(np.float32)
